# revision 9
# baseline (speedup 1.0000x reference)
"""MetaPathGNN forward on 8 Trainium2 NeuronCores (Bass/Tile) — v2.

Design (self-contained; shapes hardcoded for N=100000, C=256, OUT=128, E=400000):
  - Nodes sharded 12500/core; layer messages are RAW node features:
      layer 1: messages = x rows   -> full x table is a staged (replicated)
               ExternalInput per core, so layer 1 needs NO collective.
      layer 2: messages = h1r rows -> one AllGather of the raw LN output
               (f16) builds the shared table.
    The per-layer linear wl is applied AFTER aggregation (linearity), so the
    aggregation runs on raw features.
  - Edges owned by src core. Selector matmuls run TRANSPOSED: for each
    128-slot block of gathered messages g [slot, ch], psum[ch, node] +=
    g[:, chhalf].T @ sel[slot, node256], accumulating aggT per 2-tile group
    ("st2", 256 nodes) directly in [ch, node] orientation — no transposes.
    Self-edges bypass the gather: psum += own_features.T @ diag(mult).
  - Gathers batched: slots ordered (gg, w, st2, blk) with 10 gather-groups x
    4 dst windows -> 40 dma_gather instructions per layer.
  - Epilogue per st2: aggT -> f16, two k-halved matmuls apply wlT, then
    invdeg * aggw + z (dense term incl. biases), relu + LN-stats on ScalarE;
    batch stats -> normalize -> h (f16).
  - Dense term z per tile from persistent xT / transposed h1 with folded
    weights; overlaps gathers (no table dependency).
"""
import os
import numpy as np
from contextlib import ExitStack

N = 100000
C = 256
OUT = 128
NCORES = 8
NPC = N // NCORES          # 12500 nodes per core
P = 128
TILES = (NPC + P - 1) // P  # 98
NPC_PAD = TILES * P         # 12544
NST2 = NPC_PAD // 256       # 49 two-tile groups
ST2_PER_GG = 4              # 4 st2 per gather group -> runs are <=1024 slots
NGG = (NST2 + ST2_PER_GG - 1) // ST2_PER_GG  # 13
WIN = 25000
NWIN = 4
NFULL_PAD = 100096          # x table padded rows
LN_EPS = 1e-5

_COMPILED = {}


def _sigmoid(x):
    return 1.0 / (1.0 + np.exp(-np.float64(x)))


CHUNK_ROWS = 6250            # per-core rows in AllGather chunk A (= windows 0,1)
TBL_A = CHUNK_ROWS * NCORES  # 50000 table rows in chunk A (2 windows exactly)


def _perm_chunked(d):
    """Table position of global node d under the 2-chunk AllGather layout."""
    c, r = d // NPC, d % NPC
    a = r < CHUNK_ROWS
    return np.where(a, c * CHUNK_ROWS + r,
                    TBL_A + c * (NPC - CHUNK_ROWS) + (r - CHUNK_ROWS))


def _build_layer(src, dst, perm=None):
    """Slot/selector layout. Slot stream order: (gg, w, st2, blk).
    Returns per-core idx (wrapped), sel blob, diag blob + device meta."""
    mult = np.zeros((NCORES, NPC_PAD), dtype=np.float32)
    per_core = []
    for c in range(NCORES):
        lo = c * NPC
        m = (src >= lo) & (src < lo + NPC)
        selfm = m & (src == dst)
        si = (src[selfm] - lo).astype(np.int64)
        mult[c, :] = np.bincount(si, minlength=NPC_PAD)
        m = m & (src != dst)
        s = (src[m] - lo).astype(np.int64)
        d = dst[m].astype(np.int64)
        pos = _perm_chunked(d) if perm else d
        w = pos // WIN
        st2 = s >> 8
        gg = st2 // ST2_PER_GG
        order = np.lexsort((s, st2, w, gg))
        per_core.append((s[order], pos[order], w[order], st2[order]))

    cnt = np.zeros((NCORES, NST2, NWIN), dtype=np.int64)
    for c in range(NCORES):
        s, d, w, st2 = per_core[c]
        np.add.at(cnt[c], (st2, w), 1)
    ucnt = cnt.max(axis=0)                      # [NST2, NWIN]
    nblk = -(-ucnt // P)                        # blocks per (st2, w)

    # stream layout
    seg_slot = np.zeros((NST2, NWIN), dtype=np.int64)   # slot base per (st2,w)
    gg_meta = []      # per gg: {"runs": [(w, slot_base, ns)], "st2": {st2: [(w, blk_in_gg, nb)]}, "selbase": int, "nblk": int}
    tot_slots = 0
    tot_blocks = 0
    for gg in range(NGG):
        st2s = [t for t in range(gg * ST2_PER_GG, min((gg + 1) * ST2_PER_GG, NST2))]
        runs = []
        st2segs = {t: [] for t in st2s}
        selbase = tot_blocks
        blk_in_gg = 0
        for w in range(NWIN):
            sb = tot_slots
            for t in st2s:
                nb = int(nblk[t, w])
                if nb == 0:
                    continue
                seg_slot[t, w] = tot_slots
                st2segs[t].append((w, blk_in_gg, nb))
                tot_slots += nb * P
                tot_blocks += nb
                blk_in_gg += nb
            ns = tot_slots - sb
            if ns > 0:
                runs.append((w, sb, ns))
        gg_meta.append({"runs": runs, "st2s": st2s, "st2segs": st2segs,
                        "selbase": selbase, "nblk": blk_in_gg})

    # per-core slot placement
    idx = np.zeros((NCORES, max(tot_slots, 16)), dtype=np.int16)
    sel = np.zeros((NCORES, P, max(tot_blocks, 1) * 256), dtype=np.uint8)
    for c in range(NCORES):
        s, d, w, st2 = per_core[c]
        key = st2 * NWIN + w
        chg = np.empty(len(key), dtype=bool)
        if len(key):
            chg[0] = True
            chg[1:] = key[1:] != key[:-1]
        grp_start = np.flatnonzero(chg)
        grp_of = np.cumsum(chg) - 1
        off = np.arange(len(key)) - grp_start[grp_of]
        base = seg_slot[st2[grp_start], w[grp_start]]
        slot = base[grp_of] + off
        idx[c, slot] = (d - w * WIN).astype(np.int16)
        sel[c, slot & 127, (slot >> 7) * 256 + (s & 255)] = 1

    # wrapped idx: [128, S/16]
    S = max(tot_slots, 2048)
    S = -(-S // 2048) * 2048
    idx_full = np.zeros((NCORES, S), dtype=np.int16)
    idx_full[:, :idx.shape[1]] = idx
    ar = np.arange(S)
    idx_w = np.zeros((NCORES, 16, S // 16), dtype=np.int16)
    idx_w[:, ar % 16, ar // 16] = idx_full
    idx_w = np.tile(idx_w, (1, 8, 1))

    return {"gg": gg_meta, "total_slots": S, "total_blocks": tot_blocks,
            "idx": idx_w, "sel": sel, "mult": mult}


def _prep(inputs):
    import ml_dtypes
    f16 = np.float16
    fp8 = ml_dtypes.float8_e4m3
    x = np.asarray(inputs["x"], np.float32)
    ei1 = np.asarray(inputs["edge_index_r1"])
    ei0 = np.asarray(inputs["edge_index_r0"])

    g1 = np.float32(_sigmoid(inputs["gate1"]))
    g0 = np.float32(_sigmoid(inputs["gate0"]))
    lns1 = np.asarray(inputs["lns1"], np.float32); lnb1 = np.asarray(inputs["lnb1"], np.float32)
    lns0 = np.asarray(inputs["lns0"], np.float32); lnb0 = np.asarray(inputs["lnb0"], np.float32)
    wl1 = np.asarray(inputs["wl1"], np.float32); bl1 = np.asarray(inputs["bl1"], np.float32)
    w01 = np.asarray(inputs["w01"], np.float32); b01 = np.asarray(inputs["b01"], np.float32)
    w11 = np.asarray(inputs["w11"], np.float32); b11 = np.asarray(inputs["b11"], np.float32)
    wl0 = np.asarray(inputs["wl0"], np.float32); bl0 = np.asarray(inputs["bl0"], np.float32)
    w00 = np.asarray(inputs["w00"], np.float32); b00 = np.asarray(inputs["b00"], np.float32)
    w10 = np.asarray(inputs["w10"], np.float32); b10 = np.asarray(inputs["b10"], np.float32)
    Wout = np.asarray(inputs["Wout"], np.float32); bout = np.asarray(inputs["bout"], np.float32)

    # layer 1 (edges r1, params *1): h = x0 = x
    rhsB1 = ((1 - g1) * w01 + g1 * w11).T            # [256,256]
    brow1 = (bl1 + (1 - g1) * b01 + g1 * b11)[None]  # [1,256]
    wl1T = wl1.T                                     # post-agg
    # layer 2 (edges r0, params *0): messages h1 = lns1*h1r + lnb1
    wl0T_s = lns1[:, None] * wl0.T
    w00T_s = (1 - g0) * (lns1[:, None] * w00.T)
    w10T_s = g0 * w10.T
    crow2 = (lnb1 @ wl0.T + bl0 + (1 - g0) * (b00 + lnb1 @ w00.T) + g0 * b10)[None]
    WoutT_s = lns0[:, None] * Wout.T
    bout_s = (bout + lnb0 @ Wout.T)[None]

    inv1 = 1.0 / np.clip(np.bincount(ei1[1], minlength=N), 1.0, None).astype(np.float32)
    inv0 = 1.0 / np.clip(np.bincount(ei0[1], minlength=N), 1.0, None).astype(np.float32)

    lay1 = _build_layer(ei1[0].astype(np.int64), ei1[1].astype(np.int64))
    lay2 = _build_layer(ei0[0].astype(np.int64), ei0[1].astype(np.int64), perm=True)

    xfull = np.zeros((NFULL_PAD, C), f16)
    xfull[:N] = x.astype(f16)

    in_maps = []
    for c in range(NCORES):
        lo = c * NPC
        xs = np.zeros((NPC_PAD, C), np.float32)
        xs[:NPC] = x[lo:lo + NPC]
        inv1c = np.zeros(NPC_PAD, np.float32); inv1c[:NPC] = inv1[lo:lo + NPC]
        inv0c = np.zeros(NPC_PAD, np.float32); inv0c[:NPC] = inv0[lo:lo + NPC]
        minv1 = (lay1["mult"][c] * inv1c).reshape(TILES, P).T
        minv2 = (lay2["mult"][c] * inv0c).reshape(TILES, P).T
        in_maps.append(dict(
            xT=np.ascontiguousarray(xs.T).astype(f16),
            xfull=xfull,
            rhsB1=rhsB1.astype(f16), brow1=brow1.astype(f16), wl1T=wl1T.astype(f16),
            rhsA2=w00T_s.astype(f16), rhsY2=w10T_s.astype(f16),
            crow2=crow2.astype(f16), wl0T=wl0T_s.astype(f16),
            rhsF=WoutT_s.astype(f16), browF=bout_s.astype(f16),
            invdeg1=np.ascontiguousarray(inv1c.reshape(TILES, P).T),
            invdeg2=np.ascontiguousarray(inv0c.reshape(TILES, P).T),
            idx1=lay1["idx"][c], idx2=lay2["idx"][c],
            sel1=lay1["sel"][c].astype(np.float32).astype(fp8),
            sel2=lay2["sel"][c].astype(np.float32).astype(fp8),
            multinv1=np.ascontiguousarray(minv1),
            multinv2=np.ascontiguousarray(minv2),
        ))
    return in_maps, lay1, lay2


# ---------------------------------------------------------------- device side
def _build_nc(lay1, lay2, skip=()):
    import concourse.bass as bass
    import concourse.tile as tile
    from concourse import bacc, mybir
    from concourse.masks import make_identity

    f32, f16 = mybir.dt.float32, mybir.dt.float16
    f8, i16 = mybir.dt.float8e4, mybir.dt.int16
    AF = mybir.ActivationFunctionType
    OP = mybir.AluOpType

    nc = bacc.Bacc("TRN2", target_bir_lowering=False, debug=False, num_devices=NCORES)

    S1, S2 = lay1["total_slots"], lay2["total_slots"]
    B1, B2 = lay1["total_blocks"], lay2["total_blocks"]
    GGBLK = 0
    for lay in (lay1, lay2):
        for m in lay["gg"]:
            GGBLK = max(GGBLK, m["nblk"])

    xT_in = nc.dram_tensor("xT", [C, NPC_PAD], f16, kind="ExternalInput").ap()
    xfull_in = nc.dram_tensor("xfull", [NFULL_PAD, C], f16, kind="ExternalInput").ap()
    rhsB1_in = nc.dram_tensor("rhsB1", [C, C], f16, kind="ExternalInput").ap()
    brow1_in = nc.dram_tensor("brow1", [1, C], f16, kind="ExternalInput").ap()
    wl1T_in = nc.dram_tensor("wl1T", [C, C], f16, kind="ExternalInput").ap()
    rhsA2_in = nc.dram_tensor("rhsA2", [C, C], f16, kind="ExternalInput").ap()
    rhsY2_in = nc.dram_tensor("rhsY2", [C, C], f16, kind="ExternalInput").ap()
    crow2_in = nc.dram_tensor("crow2", [1, C], f16, kind="ExternalInput").ap()
    wl0T_in = nc.dram_tensor("wl0T", [C, C], f16, kind="ExternalInput").ap()
    rhsF_in = nc.dram_tensor("rhsF", [C, OUT], f16, kind="ExternalInput").ap()
    browF_in = nc.dram_tensor("browF", [1, OUT], f16, kind="ExternalInput").ap()
    invdeg1_in = nc.dram_tensor("invdeg1", [P, TILES], f32, kind="ExternalInput").ap()
    invdeg2_in = nc.dram_tensor("invdeg2", [P, TILES], f32, kind="ExternalInput").ap()
    idx1_in = nc.dram_tensor("idx1", [P, S1 // 16], i16, kind="ExternalInput").ap()
    idx2_in = nc.dram_tensor("idx2", [P, S2 // 16], i16, kind="ExternalInput").ap()
    sel1_in = nc.dram_tensor("sel1", [P, max(B1, 1) * 256], f8, kind="ExternalInput").ap()
    sel2_in = nc.dram_tensor("sel2", [P, max(B2, 1) * 256], f8, kind="ExternalInput").ap()
    multinv1_in = nc.dram_tensor("multinv1", [P, TILES], f32, kind="ExternalInput").ap()
    multinv2_in = nc.dram_tensor("multinv2", [P, TILES], f32, kind="ExternalInput").ap()

    out_dram = nc.dram_tensor("out", [NPC_PAD, OUT], f32, kind="ExternalOutput").ap()
    DBG = os.environ.get("K_DEBUG", "0") == "1"
    if DBG:
        dbg_z1 = nc.dram_tensor("dbg_z1", [P, TILES * C], f16, kind="ExternalOutput").ap()
        dbg_aggw1 = nc.dram_tensor("dbg_aggw1", [P, TILES * C], f32, kind="ExternalOutput").ap()
        dbg_h1 = nc.dram_tensor("dbg_h1", [P, TILES * C], f16, kind="ExternalOutput").ap()

    ag_a = nc.dram_tensor("ag_a", [CHUNK_ROWS, C], f16)
    ag_b = nc.dram_tensor("ag_b", [NPC - CHUNK_ROWS, C], f16)
    table2a = nc.dram_tensor("table2a", [TBL_A, C], f16, addr_space="Shared")
    table2b = nc.dram_tensor("table2b", [N - TBL_A, C], f16, addr_space="Shared")

    with tile.TileContext(nc) as tc, ExitStack() as ctx:
        sb = ctx.enter_context(tc.tile_pool(name="sb", bufs=1))
        xtp = ctx.enter_context(tc.tile_pool(name="xtp", bufs=2))
        gpool = ctx.enter_context(tc.tile_pool(name="gst", bufs=2))
        spool = ctx.enter_context(tc.tile_pool(name="sel", bufs=2))
        epi = ctx.enter_context(tc.tile_pool(name="epi", bufs=4))
        small = ctx.enter_context(tc.tile_pool(name="small", bufs=2))

        z_sb = sb.tile([P, TILES * C], f16)
        h_sb = sb.tile([P, TILES * C], f16)
        s1_all = sb.tile([P, TILES], f32)
        s2_all = sb.tile([P, TILES], f32)
        mu_all = sb.tile([P, TILES], f32)
        rstd_all = sb.tile([P, TILES], f32)
        invdeg1_sb = sb.tile([P, TILES], f32)
        invdeg2_sb = sb.tile([P, TILES], f32)
        multinv1_sb = sb.tile([P, TILES], f32)
        multinv2_sb = sb.tile([P, TILES], f32)
        ident16 = sb.tile([P, P], f16)
        make_identity(nc, ident16[:])
        ones_col = sb.tile([1, P], f16)
        nc.vector.memset(ones_col[:], 1.0)
        nc.sync.dma_start(invdeg1_sb[:], invdeg1_in[:])
        nc.sync.dma_start(invdeg2_sb[:], invdeg2_in[:])
        nc.sync.dma_start(multinv1_sb[:], multinv1_in[:])
        nc.sync.dma_start(multinv2_sb[:], multinv2_in[:])

        def wload(name, ap, ncols):
            t = sb.tile([P, 2, ncols], f16, tag=name)
            nc.sync.dma_start(t[:], ap.rearrange("(b k) n -> k b n", k=128))
            return t
        rhsB1_sb = wload("rhsB1", rhsB1_in[:], C)
        wl1T_sb = wload("wl1T", wl1T_in[:], C)
        rhsA2_sb = wload("rhsA2", rhsA2_in[:], C)
        rhsY2_sb = wload("rhsY2", rhsY2_in[:], C)
        wl0T_sb = wload("wl0T", wl0T_in[:], C)
        rhsF_sb = wload("rhsF", rhsF_in[:], OUT)
        brow1_sb = sb.tile([1, C], f16)
        nc.sync.dma_start(brow1_sb[:], brow1_in[:])
        crow2_sb = sb.tile([1, C], f16)
        nc.sync.dma_start(crow2_sb[:], crow2_in[:])
        browF_sb = sb.tile([1, OUT], f16)
        nc.sync.dma_start(browF_sb[:], browF_in[:])

        # max slots per gather group (for streamed idx tiles)
        GGSLOT = 0
        for lay in (lay1, lay2):
            for m in lay["gg"]:
                GGSLOT = max(GGSLOT, sum(ns for (_, _, ns) in m["runs"]))

        GT = 4  # tiles per xT load group

        def dense_chunk(layer, tlo, thi, dzps, trps, dze):
                for t0 in range(tlo, thi, GT):
                    ntl = min(GT, thi - t0)
                    xt = xtp.tile([P, 2, GT * P], f16, tag="xt")
                    nc.sync.dma_start(
                        xt[:, :, 0:ntl * P],
                        xT_in[:, t0 * P:(t0 + ntl) * P].rearrange("(b k) n -> k b n", k=128))
                    for i in range(ntl):
                        t = t0 + i
                        acc = dzps.tile([P, 2, C], f32, space="PSUM", tag="dz")
                        if layer == 1:
                            nc.tensor.matmul(acc[:, 0, :], lhsT=xt[:, 0, i * P:(i + 1) * P],
                                             rhs=rhsB1_sb[:, 0, :], start=True, stop=False)
                            nc.tensor.matmul(acc[:, 0, :], lhsT=xt[:, 1, i * P:(i + 1) * P],
                                             rhs=rhsB1_sb[:, 1, :], start=False, stop=False)
                            nc.tensor.matmul(acc[:, 0, :], lhsT=ones_col[:], rhs=brow1_sb[:],
                                             start=False, stop=False, skip_group_check=True)
                            nc.tensor.matmul(acc[:, 1, :], lhsT=xt[:, 0, i * P:(i + 1) * P],
                                             rhs=wl1T_sb[:, 0, :], start=False, stop=False,
                                             skip_group_check=True)
                            nc.tensor.matmul(acc[:, 1, :], lhsT=xt[:, 1, i * P:(i + 1) * P],
                                             rhs=wl1T_sb[:, 1, :], start=False, stop=True,
                                             skip_group_check=True)
                        else:
                            hT = dze.tile([P, 2, P], f16, tag="hT")
                            for k in range(2):
                                tp = trps.tile([P, P], f16, space="PSUM", tag="trp")
                                nc.tensor.transpose(tp[:], h_sb[:, t * C + k * P: t * C + (k + 1) * P], ident16[:])
                                nc.scalar.activation(hT[:, k, :], tp[:], AF.Copy)
                            nc.tensor.matmul(acc[:, 0, :], lhsT=hT[:, 0, :], rhs=rhsA2_sb[:, 0, :],
                                             start=True, stop=False)
                            nc.tensor.matmul(acc[:, 0, :], lhsT=hT[:, 1, :], rhs=rhsA2_sb[:, 1, :],
                                             start=False, stop=False)
                            nc.tensor.matmul(acc[:, 0, :], lhsT=xt[:, 0, i * P:(i + 1) * P],
                                             rhs=rhsY2_sb[:, 0, :], start=False, stop=False,
                                             skip_group_check=True)
                            nc.tensor.matmul(acc[:, 0, :], lhsT=xt[:, 1, i * P:(i + 1) * P],
                                             rhs=rhsY2_sb[:, 1, :], start=False, stop=False,
                                             skip_group_check=True)
                            nc.tensor.matmul(acc[:, 0, :], lhsT=ones_col[:], rhs=crow2_sb[:],
                                             start=False, stop=False, skip_group_check=True)
                            nc.tensor.matmul(acc[:, 1, :], lhsT=hT[:, 0, :], rhs=wl0T_sb[:, 0, :],
                                             start=False, stop=False, skip_group_check=True)
                            nc.tensor.matmul(acc[:, 1, :], lhsT=hT[:, 1, :], rhs=wl0T_sb[:, 1, :],
                                             start=False, stop=True, skip_group_check=True)
                        nc.scalar.activation(z_sb[:, t * C:(t + 1) * C], acc[:, 0, :], AF.Copy)
                        mi = multinv1_sb if layer == 1 else multinv2_sb
                        nc.vector.scalar_tensor_tensor(
                            out=z_sb[:, t * C:(t + 1) * C], in0=acc[:, 1, :],
                            scalar=mi[:, t:t + 1], in1=z_sb[:, t * C:(t + 1) * C],
                            op0=OP.mult, op1=OP.add)

        def dense_phase(layer):
            with tc.tile_pool(name="dzps", bufs=2, space="PSUM") as dzps, \
                 tc.tile_pool(name="trps", bufs=4, space="PSUM") as trps, \
                 tc.tile_pool(name="dze", bufs=4) as dze:
                dense_chunk(layer, 0, TILES, dzps, trps, dze)

        def gather_phase(layer, lay, tables, idx_in_ap, sel_in_ap,
                         invdeg_sb, wlT_sb, after_gg=None, inline_dense=False,
                         post_gg=None):
            if f"gp{layer}" in skip:
                if inline_dense:
                    with tc.tile_pool(name="dzps1", bufs=2, space="PSUM") as dzps:
                        dense_chunk(layer, 0, TILES, dzps, None, None)
                for t in range(TILES):
                    nc.vector.tensor_copy(h_sb[:, t * C:(t + 1) * C],
                                          z_sb[:, t * C:(t + 1) * C])
                if after_gg is not None:
                    after_gg(4)
                    after_gg(NGG - 1)
                return
            with tc.tile_pool(name="aggps", bufs=3, space="PSUM") as aggps, \
                 tc.tile_pool(name="wlps", bufs=2, space="PSUM") as wlps, \
                 ExitStack() as gctx:
                if inline_dense:
                    dzps = gctx.enter_context(
                        tc.tile_pool(name="dzps1", bufs=2, space="PSUM"))
                for gg in range(NGG):
                    meta = lay["gg"][gg]
                    nblk_gg = meta["nblk"]
                    if nblk_gg == 0:
                        continue
                    gg_sb0 = meta["runs"][0][1]    # first slot of this gg
                    gg_ns = sum(ns for (_, _, ns) in meta["runs"])
                    g_sb = gpool.tile([P, GGBLK, C], f16, tag="g")
                    sel_sb = spool.tile([P, GGBLK * 256], f8, tag="s")
                    if "gather" in skip:
                        nc.vector.memset(g_sb[:], 0.0)
                    idxg = spool.tile([P, GGSLOT // 16], i16, tag="ix")
                    nc.sync.dma_start(
                        idxg[:, 0:gg_ns // 16],
                        idx_in_ap[:, gg_sb0 // 16:(gg_sb0 + gg_ns) // 16])
                    nc.sync.dma_start(
                        sel_sb[:, 0:nblk_gg * 256],
                        sel_in_ap[:, meta["selbase"] * 256:(meta["selbase"] + nblk_gg) * 256])
                    boff = 0
                    for (w, sb0, ns) in meta["runs"]:
                        tbl, wbase = tables[w]
                        for off in range(0, ns, 1024):
                            nsc = min(1024, ns - off)
                            ib = sb0 - gg_sb0 + off
                            if "gather" not in skip:
                                nc.gpsimd.dma_gather(
                                    out_ap=g_sb[:, boff + off // P:boff + (off + nsc) // P, :],
                                    in_ap=tbl[wbase:wbase + WIN, :],
                                    idxs_ap=idxg[:, ib // 16:(ib + nsc) // 16],
                                    num_idxs=nsc, num_idxs_reg=nsc, elem_size=C,
                                )
                        boff += ns // P
                    t0 = meta["st2s"][0] * 2
                    ntl = 2 * len(meta["st2s"])
                    if inline_dense:
                        dense_chunk(layer, t0, t0 + ntl, dzps, None, None)
                    for st2 in meta["st2s"]:
                        segs = meta["st2segs"][st2]
                        nmm = 2 * sum(nb for (_, _, nb) in segs)
                        agg = aggps.tile([P, 2, 256], f32, space="PSUM", tag="agg")
                        mmi = 0
                        if "selmm" in skip:
                            for h in range(2):
                                nc.tensor.matmul(
                                    agg[:, h, :], lhsT=g_sb[:, 0, h * P:(h + 1) * P],
                                    rhs=sel_sb[:, 0:256],
                                    start=(h == 0), stop=(h == 1),
                                    skip_group_check=True)
                            segs = []
                        for (w, bog, nb) in segs:
                            for b in range(nb):
                                bb = bog + b
                                for h in range(2):
                                    if "selmm" in skip:
                                        mmi += 1
                                        continue
                                    nc.tensor.matmul(
                                        agg[:, h, :],
                                        lhsT=g_sb[:, bb, h * P:(h + 1) * P],
                                        rhs=sel_sb[:, bb * 256:(bb + 1) * 256],
                                        start=(mmi == 0), stop=(mmi == nmm - 1),
                                        skip_group_check=True)
                                    mmi += 1
                        # epilogue
                        aggT16 = epi.tile([P, 2, 256], f16, tag="a16")
                        for h in range(2):
                            nc.vector.tensor_copy(aggT16[:, h, :], agg[:, h, :])
                        for tl in range(2):
                            t = st2 * 2 + tl
                            accw = wlps.tile([P, C], f32, space="PSUM", tag="aw")
                            nc.tensor.matmul(accw[:], lhsT=aggT16[:, 0, tl * P:(tl + 1) * P],
                                             rhs=wlT_sb[:, 0, :], start=True, stop=False)
                            nc.tensor.matmul(accw[:], lhsT=aggT16[:, 1, tl * P:(tl + 1) * P],
                                             rhs=wlT_sb[:, 1, :], start=False, stop=True)
                            if DBG and layer == 1:
                                awcp = epi.tile([P, C], f32, tag="awcp")
                                nc.vector.tensor_copy(awcp[:], accw[:])
                                nc.sync.dma_start(dbg_aggw1[:, t * C:(t + 1) * C], awcp[:])
                            tmp = epi.tile([P, C], f32, tag="etmp")
                            nc.vector.scalar_tensor_tensor(
                                out=tmp[:], in0=accw[:], scalar=invdeg_sb[:, t:t + 1],
                                in1=z_sb[:, t * C:(t + 1) * C], op0=OP.mult, op1=OP.add)
                            nc.scalar.activation(z_sb[:, t * C:(t + 1) * C], tmp[:], AF.Relu,
                                                 accum_out=s1_all[:, t:t + 1])
                            sq = epi.tile([P, C], f16, tag="esq")
                            nc.scalar.activation(sq[:], z_sb[:, t * C:(t + 1) * C], AF.Square,
                                                 accum_out=s2_all[:, t:t + 1])
                    # per-gg stats -> normalize -> (layer 1) stream to ag_in2
                    sl = slice(t0, t0 + ntl)
                    nc.vector.tensor_scalar(out=mu_all[:, sl], in0=s1_all[:, sl],
                                            scalar1=1.0 / C, scalar2=None, op0=OP.mult)
                    var = small.tile([P, 10], f32, tag="var")
                    nc.vector.tensor_tensor(out=var[:, 0:ntl], in0=mu_all[:, sl],
                                            in1=mu_all[:, sl], op=OP.mult)
                    nc.vector.scalar_tensor_tensor(out=var[:, 0:ntl], in0=s2_all[:, sl],
                                                   scalar=1.0 / C, in1=var[:, 0:ntl],
                                                   op0=OP.mult, op1=OP.subtract)
                    std = small.tile([P, 10], f32, tag="std")
                    nc.vector.tensor_scalar(out=var[:, 0:ntl], in0=var[:, 0:ntl],
                                            scalar1=float(LN_EPS), scalar2=None, op0=OP.add)
                    nc.scalar.activation(std[:, 0:ntl], var[:, 0:ntl], AF.Sqrt)
                    nc.vector.reciprocal(rstd_all[:, sl], std[:, 0:ntl])
                    for t in range(t0, t0 + ntl):
                        nc.vector.tensor_scalar(
                            out=h_sb[:, t * C:(t + 1) * C], in0=z_sb[:, t * C:(t + 1) * C],
                            scalar1=mu_all[:, t:t + 1], scalar2=rstd_all[:, t:t + 1],
                            op0=OP.subtract, op1=OP.mult)
                    if layer == 1:
                        for (r0, r1, dst, dbase) in (
                                (t0 * P, min((t0 + ntl) * P, CHUNK_ROWS), ag_a, 0),
                                (max(t0 * P, CHUNK_ROWS), min((t0 + ntl) * P, NPC),
                                 ag_b, CHUNK_ROWS)):
                            t = r0 // P
                            while r0 < r1:
                                re = min(r1, (t + 1) * P)
                                # batch run of full tiles starting at tile boundary
                                if r0 == t * P:
                                    nfull = 0
                                    while (t + nfull + 1) * P <= r1:
                                        nfull += 1
                                    if nfull > 1:
                                        nc.sync.dma_start(
                                            dst[r0 - dbase:r0 - dbase + nfull * P, :].rearrange(
                                                "(b k) n -> k b n", k=128),
                                            h_sb[:, t * C:(t + nfull) * C].rearrange(
                                                "k (b n) -> k b n", n=C))
                                        r0 += nfull * P
                                        t += nfull
                                        continue
                                nc.sync.dma_start(
                                    dst[r0 - dbase:re - dbase, :],
                                    h_sb[r0 - t * P:re - t * P, t * C:(t + 1) * C])
                                r0 = re
                                t += 1
                    if after_gg is not None:
                        after_gg(gg)
                    if post_gg is not None:
                        post_gg(t0, ntl)

        # ---------------- layer 1 (no collective: table = staged xfull;
        # dense-z emission interleaved per gather group)
        def _emit_ag(which):
            nc.gpsimd.collective_compute(
                "AllGather", mybir.AluOpType.bypass,
                replica_groups=[list(range(NCORES))],
                ins=[(ag_a if which == 0 else ag_b)[:].opt()],
                outs=[(table2a if which == 0 else table2b)[:].opt()])

        def _ag_hook(gg):
            if "coll" in skip:
                return
            if gg == 6:      # chunk A rows (tiles 0..48) done after gg 6
                _emit_ag(0)
            elif gg == NGG - 1:
                _emit_ag(1)

        xtabs = tuple((xfull_in, w * WIN) for w in range(NWIN))
        gather_phase(1, lay1, xtabs, idx1_in, sel1_in,
                     invdeg1_sb, wl1T_sb, after_gg=_ag_hook, inline_dense=True)
        if DBG:
            nc.sync.dma_start(dbg_z1[:], z_sb[:])
            nc.sync.dma_start(dbg_h1[:], h_sb[:])
        # ---------------- layer 2 (collectives were emitted inside gather_phase(1))
        dense_phase(2)
        t2tabs = ((table2a, 0), (table2a, WIN), (table2b, 0), (table2b, WIN))
        # final projection emitted per gather-group right after its normalize,
        # so it overlaps the remaining groups' aggregation work
        with tc.tile_pool(name="fps", bufs=1, space="PSUM") as fps, \
             tc.tile_pool(name="ftr", bufs=2, space="PSUM") as ftr, \
             tc.tile_pool(name="fe", bufs=4) as fe:
            def final_chunk(t0, ntl):
                for t in range(t0, t0 + ntl):
                    hT = fe.tile([P, 2, P], f16, tag="fhT")
                    for k in range(2):
                        tp = ftr.tile([P, P], f16, space="PSUM", tag="ftp")
                        nc.tensor.transpose(tp[:], h_sb[:, t * C + k * P: t * C + (k + 1) * P], ident16[:])
                        nc.vector.tensor_copy(hT[:, k, :], tp[:])
                    acc = fps.tile([P, OUT], f32, space="PSUM")
                    nc.tensor.matmul(acc[:], lhsT=hT[:, 0, :], rhs=rhsF_sb[:, 0, :], start=True, stop=False)
                    nc.tensor.matmul(acc[:], lhsT=hT[:, 1, :], rhs=rhsF_sb[:, 1, :], start=False, stop=False)
                    nc.tensor.matmul(acc[:], lhsT=ones_col[:], rhs=browF_sb[:],
                                     start=False, stop=True, skip_group_check=True)
                    o_sb = fe.tile([P, OUT], f32, tag="fo")
                    nc.vector.tensor_copy(o_sb[:], acc[:])
                    nc.sync.dma_start(out_dram[t * P:(t + 1) * P, :], o_sb[:])
            gather_phase(2, lay2, t2tabs, idx2_in, sel2_in,
                         invdeg2_sb, wl0T_sb, post_gg=final_chunk)

    nc.compile()
    return nc


# ---------------------------------------------------------------- entry point
def kernel(**inputs):
    from concourse.bass_utils import run_bass_kernel_spmd

    in_maps, lay1, lay2 = _prep(inputs)
    key = "nc"
    if key not in _COMPILED:
        _COMPILED[key] = _build_nc(lay1, lay2)
    nc = _COMPILED[key]
    res = run_bass_kernel_spmd(nc, in_maps, core_ids=list(range(NCORES)))
    _COMPILED["last_res"] = res
    out = np.concatenate([res.results[c]["out"][:NPC] for c in range(NCORES)], axis=0)
    return out.astype(np.float32)
